# revision 6
# baseline (speedup 1.0000x reference)
"""Trainium2 Bass kernel for a 12-layer BERT-style transformer encoder stack.

Reference computation (per layer):
    q,k,v = x@Wq+bq, x@Wk+bk, x@Wv+bv          (x: [S,B,H])
    attn  = softmax(q@k^T / sqrt(HD)) @ v       (per (batch, head))
    x     = LayerNorm(attn@Wo + bo + x) * gamma + beta

Sharding (8 cores): 2-way batch data-parallel x 4-way head tensor-parallel
(Megatron).  Core c handles batch c//4 and heads [4*(c%4), 4*(c%4)+4).
Wq/Wk/Wv are column-sliced, Wo row-sliced; the per-layer partial outputs
(ctx @ Wo_slice) are AllReduce'd within each 4-core quad, chunked by
sequence quarters so communication overlaps attention compute.

On-chip layout: everything lives feature-major ("transposed", [H, S]) so
that the PE contraction dim (partitions) is always the feature dim and no
on-chip transposes are ever needed.  LayerNorm statistics over the feature
(partition) dim are computed with ones-vector matmuls; per-sequence scalars
are broadcast across partitions on the (otherwise idle) GpSimd engine.
Attention processes head PAIRS interleaved: the two heads of a pair sit at
partition offsets 0/64 of the same qT/kT tile, so their K=64 score matmuls
land on disjoint PE row-groups and execute concurrently.  The next layer's
q/k/v projections are emitted before the last LayerNorm quarter so the
final AllReduce's latency is hidden behind useful PE work.  Matmul inputs
are fp16, accumulation fp32.
"""

import sys

sys.path.insert(0, "/opt/trn_rl_repo")

import numpy as np

import concourse.bass as bass
import concourse.tile as tile
from concourse import bacc
from concourse import mybir
from concourse.bass_utils import run_bass_kernel_spmd

# Problem constants
S, B, H, NH, L = 2048, 2, 1024, 16, 12
HD = H // NH          # 64
EPS = 1e-12
N_CORES = 8
NHL = 4               # heads per core (4-way head split)
DQ = NHL * HD         # 256 local feature cols for q/k/v
HC = H // 128         # 8 h-chunks of 128 partitions
MQ = DQ // 128        # 2 local m-chunks

F16 = mybir.dt.float16
F32 = mybir.dt.float32

REPLICA_GROUPS = [[0, 1, 2, 3], [4, 5, 6, 7]]


def build_bass(s=S, l_layers=L, quads=REPLICA_GROUPS,
               fast_recip=True, gp_bcast=True):
    """Builds the SPMD Bass program (identical on all 8 cores)."""
    QW = s // 4            # sequence quarter width (AR chunk) <= 512
    NT = s // 128          # 128-row t-chunks of the sequence
    NTP = NT // 2          # t-chunk pairs share one 2-bank psum + one Exp
    LAG = 1
    assert QW <= 512 and s % 128 == 0

    nc = bacc.Bacc("TRN2", num_devices=N_CORES)

    # ---- I/O ----
    xT0 = nc.dram_tensor("xT0", [HC, 128, s], F16, kind="ExternalInput")
    wq_d = nc.dram_tensor("wq", [l_layers, 128, HC, DQ], F16, kind="ExternalInput")
    wk_d = nc.dram_tensor("wk", [l_layers, 128, HC, DQ], F16, kind="ExternalInput")
    wv_d = nc.dram_tensor("wv", [l_layers, 128, HC, DQ], F16, kind="ExternalInput")
    wo_d = nc.dram_tensor("wo", [l_layers, 128, MQ, H], F16, kind="ExternalInput")
    bqk_d = nc.dram_tensor("bqk", [l_layers, 128, 2 * MQ], F32, kind="ExternalInput")
    lnw_d = nc.dram_tensor("lnw", [l_layers, 128, HC, 3], F32, kind="ExternalInput")
    outx = nc.dram_tensor("outx", [HC, 128, s], F32, kind="ExternalOutput")

    from contextlib import ExitStack

    with tile.TileContext(nc) as tc:
        with ExitStack() as ctx:
            pool = lambda *a, **kw: ctx.enter_context(tc.tile_pool(*a, **kw))
            consts = pool(name="consts", bufs=1)
            xTp = pool(name="xT", bufs=HC)
            w3p = pool(name="w3", bufs=6)          # wq/wk/wv x 2 layers
            wop = pool(name="wo", bufs=2)
            smallp = pool(name="small", bufs=2)
            qkp = pool(name="qkT", bufs=9)         # qT/kT x 2 layers + 1
            ctxp = pool(name="ctxT", bufs=3)
            vp = pool(name="vsb", bufs=NT + 1)
            prp = pool(name="probs", bufs=5)
            otp = pool(name="outT", bufs=2 * HC)   # per-(quarter, chunk)
            dsp = pool(name="dsend", bufs=8)
            sqp = pool(name="sq", bufs=2)
            ltp = pool(name="lntmp", bufs=2)
            lrp = pool(name="lnrow", bufs=6)
            rrp = pool(name="rrow", bufs=2)
            fop = pool(name="fout", bufs=2)
            # PSUM: ss 2x2 banks + pb 2x1 + pa 2x1 = 8 banks exactly.
            pa = pool(name="pa", bufs=2, space="PSUM")
            pb = pool(name="pb", bufs=2, space="PSUM")
            ps2 = pool(name="ps2", bufs=2, space="PSUM")
            dramp = pool(name="dram", bufs=16, space="DRAM")

            ones16 = consts.tile([128, 128], F16, tag="ones16")
            nc.vector.memset(ones16[:], 1.0)
            eps_sb = consts.tile([128, 1], F32, tag="eps")
            nc.vector.memset(eps_sb[:], EPS)

            # Persistent x^T state (fp16), one tile per 128-feature chunk.
            xT = []
            for c in range(HC):
                t = xTp.tile([128, s], F16, tag="xT", name=f"xT{c}")
                nc.sync.dma_start(t[:], xT0[c, :, :])
                xT.append(t)

            # ---- per-layer weight state ----
            W = {}

            def load_weights(l):
                wq_sb = w3p.tile([128, HC, DQ], F16, tag="w3", name=f"wq{l}")
                wk_sb = w3p.tile([128, HC, DQ], F16, tag="w3", name=f"wk{l}")
                wv_sb = w3p.tile([128, HC, DQ], F16, tag="w3", name=f"wv{l}")
                nc.sync.dma_start(wq_sb[:], wq_d[l, :, :, :])
                nc.sync.dma_start(wk_sb[:], wk_d[l, :, :, :])
                nc.sync.dma_start(wv_sb[:], wv_d[l, :, :, :])
                wo_sb = wop.tile([128, MQ, H], F16, tag="wo", name=f"wo{l}")
                nc.sync.dma_start(wo_sb[:], wo_d[l, :, :, :])
                bqk_sb = smallp.tile([128, 2 * MQ], F32, tag="bqk", name=f"bqk{l}")
                nc.sync.dma_start(bqk_sb[:], bqk_d[l, :, :])
                lnw_sb = smallp.tile([128, HC, 3], F32, tag="lnw", name=f"lnw{l}")
                nc.sync.dma_start(lnw_sb[:], lnw_d[l, :, :, :])
                W[l] = dict(wq=wq_sb, wk=wk_sb, wv=wv_sb, wo=wo_sb,
                            bqk=bqk_sb, lnw=lnw_sb)

            # ---- projection state (qT/kT/v per layer) ----
            QK = {}
            V = {}

            def proj_qk_quarter(l, qi):
                """q^T,k^T for quarter qi of layer l: [DQ, QW] slices."""
                if l not in QK:
                    qT = [qkp.tile([128, s], F16, tag="qkT", name=f"qT{l}_{m}")
                          for m in range(MQ)]
                    kT = [qkp.tile([128, s], F16, tag="qkT", name=f"kT{l}_{m}")
                          for m in range(MQ)]
                    QK[l] = (qT, kT)
                qT, kT = QK[l]
                w = W[l]
                sw = slice(qi * QW, (qi + 1) * QW)
                for m in range(MQ):
                    for dst, w_sb, bcol in ((qT, w["wq"], m), (kT, w["wk"], MQ + m)):
                        ps = pa.tile([128, QW], F32, tag="pa")
                        for c in range(HC):
                            nc.tensor.matmul(
                                ps[:],
                                w_sb[:, c, m * 128:(m + 1) * 128],
                                xT[c][:, sw],
                                start=(c == 0),
                                stop=(c == HC - 1),
                            )
                        nc.scalar.activation(
                            out=dst[m][:, sw],
                            in_=ps[:],
                            func=mybir.ActivationFunctionType.Identity,
                            bias=w["bqk"][:, bcol:bcol + 1],
                        )

            def proj_v_chunks(l, trange):
                """v tiles [t, d] with a ones column per head, for t in trange."""
                if l not in V:
                    V[l] = {}
                w = W[l]
                for t in trange:
                    vt = vp.tile([128, NHL, HD + 1], F16, tag="vsb",
                                 name=f"v{l}_{t}")
                    ps = pa.tile([128, max(QW, DQ)], F32, tag="pa")
                    for c in range(HC):
                        nc.tensor.matmul(
                            ps[:, 0:DQ],
                            xT[c][:, t * 128:(t + 1) * 128],
                            w["wv"][:, c, :],
                            start=(c == 0),
                            stop=(c == HC - 1),
                        )
                    nc.vector.tensor_copy(
                        out=vt[:, :, 0:HD],
                        in_=ps[:, 0:DQ].rearrange("p (h d) -> p h d", h=NHL),
                    )
                    nc.vector.memset(vt[:, :, HD:HD + 1], 1.0)
                    V[l][t] = vt

            # ---- per-layer transient state ----
            arouts = {}
            CTX = {}

            def attn_quarter(l, qi):
                """Attention for quarter qi, head pairs interleaved."""
                sw = slice(qi * QW, (qi + 1) * QW)
                qT, kT = QK[l]
                if l not in CTX:
                    CTX[l] = [ctxp.tile([128, s], F16, tag="ctxT",
                                        name=f"ctxT{l}_{m}") for m in range(MQ)]
                ctxT = CTX[l]
                for m in range(MQ):          # pair m: heads 2m (off 0), 2m+1 (off 64)
                    heads = (2 * m, 2 * m + 1)
                    pctx = {}
                    probs = {h: [None] * NTP for h in heads}

                    def ctx_mm(h, tp):
                        for half in range(2):
                            t = 2 * tp + half
                            nc.tensor.matmul(
                                pctx[h][:],
                                V[l][t][:, h, :],
                                probs[h][tp][:, half * QW:(half + 1) * QW],
                                start=(t == 0),
                                stop=(t == NT - 1),
                            )

                    for h in heads:
                        pctx[h] = pb.tile([65, QW], F32, tag="pb",
                                          name=f"pctx{l}_{qi}_{h}")
                    for tp in range(NTP):
                        for h in heads:
                            off = 64 * (h % 2)
                            ss = ps2.tile([128, 2 * QW], F32, tag="ss",
                                          name=f"ss{l}_{qi}_{h}_{tp}")
                            for half in range(2):
                                t = 2 * tp + half
                                nc.tensor.matmul(
                                    ss[:, half * QW:(half + 1) * QW],
                                    kT[m][off:off + 64, t * 128:(t + 1) * 128],
                                    qT[m][off:off + 64, sw],
                                    start=True,
                                    stop=True,
                                )
                            pr = prp.tile([128, 2 * QW], F16, tag="probs",
                                          name=f"pr{l}_{qi}_{h}_{tp}")
                            nc.scalar.activation(
                                out=pr[:],
                                in_=ss[:],
                                func=mybir.ActivationFunctionType.Exp,
                                scale=float(1.0 / np.sqrt(HD)),
                            )
                            probs[h][tp] = pr
                        if tp >= LAG:
                            for h in heads:
                                ctx_mm(h, tp - LAG)
                    for tp in range(NTP - LAG, NTP):
                        for h in heads:
                            ctx_mm(h, tp)

                    # normalize: ctx^T[d, s'] * (1 / l[s']), l at psum row 64
                    for h in heads:
                        off = 64 * (h % 2)
                        if gp_bcast:
                            r_sb = lrp.tile([1, QW], F32, tag="lnrow",
                                            name=f"r{l}_{qi}_{h}")
                            if fast_recip:
                                nc.vector.reciprocal_approx_fast(
                                    out=r_sb[:], in_=pctx[h][64:65, :]
                                )
                            else:
                                nc.vector.reciprocal(r_sb[:], pctx[h][64:65, :])
                            bcs = rrp.tile([64, QW], F32, tag="bcs",
                                           name=f"bcs{l}_{qi}_{h}")
                            nc.gpsimd.partition_broadcast(
                                bcs[:], r_sb[:], channels=64
                            )
                        else:
                            r_sb = lrp.tile([1, QW], F16, tag="lnrow16",
                                            name=f"r{l}_{qi}_{h}")
                            with nc.allow_low_precision(reason="softmax denom"):
                                nc.vector.reciprocal(r_sb[:], pctx[h][64:65, :])
                            bc = pa.tile([128, max(QW, DQ)], F32, tag="pa")
                            nc.tensor.matmul(
                                bc[0:64, 0:QW], ones16[0:1, 0:64], r_sb[:],
                                start=True, stop=True,
                            )
                            bcs = rrp.tile([64, QW], F16, tag="bcs16",
                                           name=f"bcs{l}_{qi}_{h}")
                            nc.vector.tensor_copy(out=bcs[:], in_=bc[0:64, 0:QW])
                        nc.vector.tensor_mul(
                            out=ctxT[m][off:off + 64, sw],
                            in0=pctx[h][0:64, :],
                            in1=bcs[:],
                        )

            def emit_delta_ar(l, qj):
                # Wo partials for quarter qj -> DRAM bounce -> quad AllReduce
                swj = slice(qj * QW, (qj + 1) * QW)
                ctxT = CTX[l]
                wo_sb = W[l]["wo"]
                arin = dramp.tile([HC, 128, QW], F16, tag="arin",
                                  name=f"arin{l}_{qj}")
                arout = dramp.tile([HC, 128, QW], F16, tag="arout",
                                   name=f"arout{l}_{qj}")
                for c in range(HC):
                    pd = pa.tile([128, QW], F32, tag="pa", name=f"pd{l}_{qj}_{c}")
                    for m in range(MQ):
                        nc.tensor.matmul(
                            pd[:],
                            wo_sb[:, m, c * 128:(c + 1) * 128],
                            ctxT[m][:, swj],
                            start=(m == 0),
                            stop=(m == MQ - 1),
                        )
                    ds = dsp.tile([128, QW], F16, tag="dsend",
                                  name=f"ds{l}_{qj}_{c}")
                    nc.vector.tensor_copy(out=ds[:], in_=pd[:])
                    nc.sync.dma_start(arin[c, :, :], ds[:])
                nc.gpsimd.collective_compute(
                    "AllReduce",
                    mybir.AluOpType.add,
                    replica_groups=quads,
                    ins=[arin[:].opt()],
                    outs=[arout[:].opt()],
                )
                arouts[(l, qj)] = arout

            def ln_quarter(l, qi):
                """out^T = AR(delta) + bo_eff + x^T ; LN; update x^T."""
                last = l == l_layers - 1
                sw = slice(qi * QW, (qi + 1) * QW)
                arout = arouts[(l, qi)]
                lnw_sb = W[l]["lnw"]
                pst = pb.tile([65, QW], F32, tag="pb", name=f"pst{l}_{qi}")
                ots = []
                for c in range(HC):
                    ot = otp.tile([128, QW], F16, tag="outT",
                                  name=f"ot{l}_{qi}_{c}")
                    nc.sync.dma_start(ot[:], arout[c, :, :])
                    nc.vector.scalar_tensor_tensor(
                        out=ot[:],
                        in0=ot[:],
                        scalar=lnw_sb[:, c, 2:3],
                        in1=xT[c][:, sw],
                        op0=mybir.AluOpType.add,
                        op1=mybir.AluOpType.add,
                    )
                    sqt = sqp.tile([128, QW], F16, tag="sq")
                    nc.vector.tensor_mul(out=sqt[:], in0=ot[:], in1=ot[:])
                    nc.tensor.matmul(
                        pst[0:1, :], ones16[:, 0:1], ot[:],
                        start=(c == 0), stop=(c == HC - 1),
                        skip_group_check=True,
                    )
                    nc.tensor.matmul(
                        pst[32:33, :], ones16[:, 0:1], sqt[:],
                        start=(c == 0), stop=(c == HC - 1),
                        skip_group_check=True,
                    )
                    ots.append(ot)
                sumx = lrp.tile([1, QW], F32, tag="lnrow", name=f"sx{l}_{qi}")
                sumsq = lrp.tile([1, QW], F32, tag="lnrow", name=f"sq{l}_{qi}")
                nc.vector.tensor_copy(out=sumx[:], in_=pst[0:1, :])
                nc.vector.tensor_copy(out=sumsq[:], in_=pst[32:33, :])

                m_sb = lrp.tile([1, QW], F32, tag="lnrow", name=f"m{l}_{qi}")
                nc.vector.tensor_scalar_mul(m_sb[:], sumx[:], 1.0 / H)
                m2 = lrp.tile([1, QW], F32, tag="lnrow", name=f"m2{l}_{qi}")
                nc.vector.tensor_mul(m2[:], m_sb[:], m_sb[:])
                var = lrp.tile([1, QW], F32, tag="lnrow", name=f"va{l}_{qi}")
                nc.vector.scalar_tensor_tensor(
                    out=var[:], in0=sumsq[:], scalar=1.0 / H, in1=m2[:],
                    op0=mybir.AluOpType.mult, op1=mybir.AluOpType.subtract,
                )
                sd = lrp.tile([1, QW], F32, tag="lnrow", name=f"sd{l}_{qi}")
                nc.scalar.activation(
                    out=sd[:], in_=var[:],
                    func=mybir.ActivationFunctionType.Sqrt,
                    bias=eps_sb[0:1, :],
                )
                rstd = lrp.tile([1, QW], F32, tag="lnrow", name=f"rs{l}_{qi}")
                if fast_recip:
                    nc.vector.reciprocal_approx_fast(out=rstd[:], in_=sd[:])
                else:
                    nc.vector.reciprocal(rstd[:], sd[:])

                # broadcast stats across partitions, apply, update x^T
                if gp_bcast:
                    mb = rrp.tile([128, QW], F32, tag="mrb", name=f"mb{l}_{qi}")
                    nc.gpsimd.partition_broadcast(mb[:], m_sb[:], channels=128)
                    rb = rrp.tile([128, QW], F32, tag="mrb", name=f"rb{l}_{qi}")
                    nc.gpsimd.partition_broadcast(rb[:], rstd[:], channels=128)
                else:
                    m16 = lrp.tile([1, QW], F16, tag="lnrow16", name=f"m16{l}_{qi}")
                    nc.vector.tensor_copy(out=m16[:], in_=m_sb[:])
                    r16 = lrp.tile([1, QW], F16, tag="lnrow16", name=f"r16{l}_{qi}")
                    nc.vector.tensor_copy(out=r16[:], in_=rstd[:])
                    mb = pa.tile([128, max(QW, DQ)], F32, tag="pa",
                                 name=f"mb{l}_{qi}")
                    nc.tensor.matmul(
                        mb[:, 0:QW], ones16[0:1, :], m16[:], start=True, stop=True
                    )
                    mb = mb[:, 0:QW]
                    rb = pa.tile([128, max(QW, DQ)], F32, tag="pa",
                                 name=f"rb{l}_{qi}")
                    nc.tensor.matmul(
                        rb[:, 0:QW], ones16[0:1, :], r16[:], start=True, stop=True
                    )
                    rb = rb[:, 0:QW]
                for c in range(HC):
                    tmp = ltp.tile([128, QW], F32, tag="lntmp")
                    nc.vector.tensor_sub(out=tmp[:], in0=ots[c][:], in1=mb[:])
                    nc.vector.scalar_tensor_tensor(
                        out=tmp[:], in0=tmp[:],
                        scalar=lnw_sb[:, c, 0:1], in1=rb[:],
                        op0=mybir.AluOpType.mult, op1=mybir.AluOpType.mult,
                    )
                    if last:
                        fo = fop.tile([128, QW], F32, tag="fout")
                        nc.vector.tensor_scalar_add(
                            fo[:], tmp[:], lnw_sb[:, c, 1:2]
                        )
                        nc.sync.dma_start(outx[c, :, sw], fo[:])
                    else:
                        nc.vector.tensor_scalar_add(
                            xT[c][:, sw], tmp[:], lnw_sb[:, c, 1:2]
                        )

            # ---- schedule ----
            load_weights(0)
            for qi in range(4):
                proj_qk_quarter(0, qi)
            proj_v_chunks(0, range(NT))

            for l in range(l_layers):
                last = l == l_layers - 1
                if not last:
                    load_weights(l + 1)
                for qi in range(4):
                    attn_quarter(l, qi)
                    emit_delta_ar(l, qi)
                    if qi >= 1:
                        ln_quarter(l, qi - 1)
                # tail: next layer's projections fill the last AR's latency
                if not last:
                    for qi in range(3):
                        proj_qk_quarter(l + 1, qi)
                    proj_v_chunks(l + 1, range(12))
                ln_quarter(l, 3)
                if not last:
                    proj_qk_quarter(l + 1, 3)
                    proj_v_chunks(l + 1, range(12, NT))
                # drop dead references so pools can recycle
                for d in (QK, V, CTX, W):
                    d.pop(l, None)
    nc.compile()
    return nc


def make_in_maps(inputs, s=S, l_layers=L):
    """Host-side sharding: returns one input dict per core."""
    x = np.asarray(inputs["input_tensor"], dtype=np.float32)      # [s, B, H]
    Wq = np.asarray(inputs["Wq"], dtype=np.float32)[:l_layers]
    Wk = np.asarray(inputs["Wk"], dtype=np.float32)[:l_layers]
    Wv = np.asarray(inputs["Wv"], dtype=np.float32)[:l_layers]
    Wo = np.asarray(inputs["Wo"], dtype=np.float32)[:l_layers]
    bq = np.asarray(inputs["bq"], dtype=np.float32)[:l_layers]
    bk = np.asarray(inputs["bk"], dtype=np.float32)[:l_layers]
    bv = np.asarray(inputs["bv"], dtype=np.float32)[:l_layers]
    bo = np.asarray(inputs["bo"], dtype=np.float32)[:l_layers]
    gamma = np.asarray(inputs["gamma"], dtype=np.float32)[:l_layers]
    beta = np.asarray(inputs["beta"], dtype=np.float32)[:l_layers]
    ll = l_layers

    # bv passes through the softmax-weighted sum exactly: fold bv@Wo into bo.
    bo_eff = bo + np.einsum("lh,lhk->lk", bv, Wo)

    def chunkP(a, n_out):
        # [..., n_out*128, inner] -> [..., 128, n_out, inner] feature-chunked
        sh = a.shape
        a = a.reshape(*sh[:-2], n_out, 128, sh[-1])
        return np.moveaxis(a, -3, -2)  # -> [..., 128, n_out, inner]

    in_maps = []
    for core in range(N_CORES):
        g, j = core // 4, core % 4
        cols = slice(DQ * j, DQ * (j + 1))
        xT = np.ascontiguousarray(x[:, g, :].T).reshape(HC, 128, s)
        wq = np.ascontiguousarray(chunkP(Wq[:, :, cols], HC))      # [L,128,HC,DQ]
        wk = np.ascontiguousarray(chunkP(Wk[:, :, cols], HC))
        wv = np.ascontiguousarray(chunkP(Wv[:, :, cols], HC))
        wo = np.ascontiguousarray(chunkP(Wo[:, cols, :], MQ))      # [L,128,MQ,H]
        bqs = bq[:, cols].reshape(ll, MQ, 128).transpose(0, 2, 1)  # [L,128,MQ]
        bks = bk[:, cols].reshape(ll, MQ, 128).transpose(0, 2, 1)
        bqk = np.ascontiguousarray(np.concatenate([bqs, bks], axis=2))
        lnw = np.stack(
            [
                gamma.reshape(ll, HC, 128).transpose(0, 2, 1),
                beta.reshape(ll, HC, 128).transpose(0, 2, 1),
                bo_eff.reshape(ll, HC, 128).transpose(0, 2, 1),
            ],
            axis=3,
        )                                                          # [L,128,HC,3]
        in_maps.append(
            {
                "xT0": xT.astype(np.float16),
                "wq": wq.astype(np.float16),
                "wk": wk.astype(np.float16),
                "wv": wv.astype(np.float16),
                "wo": wo.astype(np.float16),
                "bqk": bqk.astype(np.float32),
                "lnw": np.ascontiguousarray(lnw).astype(np.float32),
            }
        )
    return in_maps


_NC_CACHE = {}


def kernel(**inputs) -> np.ndarray:
    in_maps = make_in_maps(inputs)
    key = (S, L)
    if key not in _NC_CACHE:
        _NC_CACHE[key] = build_bass()
    nc = _NC_CACHE[key]
    res = run_bass_kernel_spmd(nc, in_maps, core_ids=list(range(N_CORES)))
    out = np.empty((S, B, H), dtype=np.float32)
    for g, core in ((0, 0), (1, 4)):
        xt = res.results[core]["outx"].reshape(H, S)
        out[:, g, :] = xt.T
    return out


# revision 12
# speedup vs baseline: 1.1390x; 1.1390x over previous
"""Trainium2 Bass kernel for a 12-layer BERT-style transformer encoder stack.

Reference computation (per layer):
    q,k,v = x@Wq+bq, x@Wk+bk, x@Wv+bv          (x: [S,B,H])
    attn  = softmax(q@k^T / sqrt(HD)) @ v       (per (batch, head))
    x     = LayerNorm(attn@Wo + bo + x) * gamma + beta

Sharding (8 cores): 2-way batch data-parallel x 4-way head tensor-parallel
(Megatron).  Core c handles batch c//4 and heads [4*(c%4), 4*(c%4)+4).
Wq/Wk/Wv are column-sliced, Wo row-sliced; the per-layer partial outputs
(ctx @ Wo_slice) are AllReduce'd within each 4-core quad, chunked by
sequence quarters so communication overlaps attention compute.

On-chip layout: everything lives feature-major ("transposed", [H, S]) so
that the PE contraction dim (partitions) is always the feature dim and no
on-chip transposes are ever needed.  LayerNorm statistics over the feature
(partition) dim are computed with ones-vector matmuls; per-sequence scalars
are broadcast across partitions on the (otherwise idle) GpSimd engine.
Attention processes head PAIRS interleaved: the two heads of a pair sit at
partition offsets 0/64 of the same qT/kT tile, so their K=64 score matmuls
land on disjoint PE row-groups and execute concurrently.  The next layer's
q/k/v projections are emitted before the last LayerNorm quarter so the
final AllReduce's latency is hidden behind useful PE work.  Matmul inputs
are fp16, accumulation fp32.
"""

import sys

sys.path.insert(0, "/opt/trn_rl_repo")

import numpy as np

import concourse.bass as bass
import concourse.tile as tile
from concourse import bacc
from concourse import mybir
from concourse.bass_utils import run_bass_kernel_spmd

# Problem constants
S, B, H, NH, L = 2048, 2, 1024, 16, 12
HD = H // NH          # 64
EPS = 1e-12
N_CORES = 8
NHL = 4               # heads per core (4-way head split)
DQ = NHL * HD         # 256 local feature cols for q/k/v
HC = H // 128         # 8 h-chunks of 128 partitions
MQ = DQ // 128        # 2 local m-chunks

F16 = mybir.dt.float16
F32 = mybir.dt.float32

REPLICA_GROUPS = [[0, 1, 2, 3], [4, 5, 6, 7]]


def build_bass(s=S, l_layers=L, quads=REPLICA_GROUPS,
               fast_recip=True, gp_bcast=True):
    """Builds the SPMD Bass program (identical on all 8 cores)."""
    QW = s // 4            # sequence quarter width (AR chunk) <= 512
    NT = s // 128          # 128-row t-chunks of the sequence
    NTP = NT // 2          # t-chunk pairs share one 2-bank psum + one Exp
    LAG = 1
    assert QW <= 512 and s % 128 == 0

    nc = bacc.Bacc("TRN2", num_devices=N_CORES)

    # ---- I/O ----
    xT0 = nc.dram_tensor("xT0", [HC, 128, s], F16, kind="ExternalInput")
    wq_d = nc.dram_tensor("wq", [l_layers, 128, HC, DQ], F16, kind="ExternalInput")
    wk_d = nc.dram_tensor("wk", [l_layers, 128, HC, DQ], F16, kind="ExternalInput")
    wv_d = nc.dram_tensor("wv", [l_layers, 128, HC, DQ], F16, kind="ExternalInput")
    wo_d = nc.dram_tensor("wo", [l_layers, 128, MQ, H], F16, kind="ExternalInput")
    bqk_d = nc.dram_tensor("bqk", [l_layers, 128, 2 * MQ], F32, kind="ExternalInput")
    lnw_d = nc.dram_tensor("lnw", [l_layers, 128, HC, 3], F32, kind="ExternalInput")
    outx = nc.dram_tensor("outx", [HC, 128, s], F32, kind="ExternalOutput")

    from contextlib import ExitStack

    with tile.TileContext(nc) as tc:
        with ExitStack() as ctx:
            pool = lambda *a, **kw: ctx.enter_context(tc.tile_pool(*a, **kw))
            consts = pool(name="consts", bufs=1)
            xTp = pool(name="xT", bufs=HC)
            w3p = pool(name="w3", bufs=6)          # wq/wk/wv x 2 layers
            wop = pool(name="wo", bufs=2)
            smallp = pool(name="small", bufs=2)
            qkp = pool(name="qkT", bufs=9)         # qT/kT x 2 layers + 1
            ctxp = pool(name="ctxT", bufs=3)
            vp = pool(name="vsb", bufs=NT + 1)
            prp = pool(name="probs", bufs=5)
            otp = pool(name="outT", bufs=2 * HC)   # per-(quarter, chunk)
            dsp = pool(name="dsend", bufs=8)
            sqp = pool(name="sq", bufs=2)
            ltp = pool(name="lntmp", bufs=2)
            lrp = pool(name="lnrow", bufs=6)
            rrp = pool(name="rrow", bufs=2)
            fop = pool(name="fout", bufs=2)
            # PSUM: ss 2x2 banks + pb 2x1 + pa 2x1 = 8 banks exactly.
            pa = pool(name="pa", bufs=2, space="PSUM")
            pb = pool(name="pb", bufs=2, space="PSUM")
            ps2 = pool(name="ps2", bufs=2, space="PSUM")
            dramp = pool(name="dram", bufs=16, space="DRAM")

            ones16 = consts.tile([128, 128], F16, tag="ones16")
            nc.vector.memset(ones16[:], 1.0)
            eps_sb = consts.tile([128, 1], F32, tag="eps")
            nc.vector.memset(eps_sb[:], EPS)

            # Persistent x^T state (fp16), one tile per 128-feature chunk.
            xT = []
            for c in range(HC):
                t = xTp.tile([128, s], F16, tag="xT", name=f"xT{c}")
                nc.sync.dma_start(t[:], xT0[c, :, :])
                xT.append(t)

            # ---- per-layer weight state ----
            W = {}

            def load_weights(l):
                wq_sb = w3p.tile([128, HC, DQ], F16, tag="w3", name=f"wq{l}")
                wk_sb = w3p.tile([128, HC, DQ], F16, tag="w3", name=f"wk{l}")
                wv_sb = w3p.tile([128, HC, DQ], F16, tag="w3", name=f"wv{l}")
                nc.sync.dma_start(wq_sb[:], wq_d[l, :, :, :])
                nc.sync.dma_start(wk_sb[:], wk_d[l, :, :, :])
                nc.sync.dma_start(wv_sb[:], wv_d[l, :, :, :])
                wo_sb = wop.tile([128, MQ, H], F16, tag="wo", name=f"wo{l}")
                nc.sync.dma_start(wo_sb[:], wo_d[l, :, :, :])
                bqk_sb = smallp.tile([128, 2 * MQ], F32, tag="bqk", name=f"bqk{l}")
                nc.sync.dma_start(bqk_sb[:], bqk_d[l, :, :])
                lnw_sb = smallp.tile([128, HC, 3], F32, tag="lnw", name=f"lnw{l}")
                nc.sync.dma_start(lnw_sb[:], lnw_d[l, :, :, :])
                W[l] = dict(wq=wq_sb, wk=wk_sb, wv=wv_sb, wo=wo_sb,
                            bqk=bqk_sb, lnw=lnw_sb)

            # ---- projection state (qT/kT/v per layer) ----
            QK = {}
            V = {}

            def proj_qk_quarter(l, qi):
                """q^T,k^T for quarter qi of layer l: [DQ, QW] slices."""
                if l not in QK:
                    qT = [qkp.tile([128, s], F16, tag="qkT", name=f"qT{l}_{m}")
                          for m in range(MQ)]
                    kT = [qkp.tile([128, s], F16, tag="qkT", name=f"kT{l}_{m}")
                          for m in range(MQ)]
                    QK[l] = (qT, kT)
                qT, kT = QK[l]
                w = W[l]
                sw = slice(qi * QW, (qi + 1) * QW)
                for m in range(MQ):
                    for dst, w_sb, bcol in ((qT, w["wq"], m), (kT, w["wk"], MQ + m)):
                        ps = pa.tile([128, QW], F32, tag="pa")
                        for c in range(HC):
                            nc.tensor.matmul(
                                ps[:],
                                w_sb[:, c, m * 128:(m + 1) * 128],
                                xT[c][:, sw],
                                start=(c == 0),
                                stop=(c == HC - 1),
                            )
                        nc.scalar.activation(
                            out=dst[m][:, sw],
                            in_=ps[:],
                            func=mybir.ActivationFunctionType.Identity,
                            bias=w["bqk"][:, bcol:bcol + 1],
                        )

            def proj_v_chunks(l, trange):
                """v tiles [t, d] with a ones column per head, for t in trange."""
                if l not in V:
                    V[l] = {}
                w = W[l]
                for t in trange:
                    vt = vp.tile([128, NHL, HD + 1], F16, tag="vsb",
                                 name=f"v{l}_{t}")
                    ps = pa.tile([128, max(QW, DQ)], F32, tag="pa")
                    for c in range(HC):
                        nc.tensor.matmul(
                            ps[:, 0:DQ],
                            xT[c][:, t * 128:(t + 1) * 128],
                            w["wv"][:, c, :],
                            start=(c == 0),
                            stop=(c == HC - 1),
                        )
                    nc.vector.tensor_copy(
                        out=vt[:, :, 0:HD],
                        in_=ps[:, 0:DQ].rearrange("p (h d) -> p h d", h=NHL),
                    )
                    nc.vector.memset(vt[:, :, HD:HD + 1], 1.0)
                    V[l][t] = vt

            # ---- per-layer transient state ----
            arouts = {}
            CTX = {}

            def attn_quarter(l, qi, fillers=None):
                """Attention for quarter qi, head pairs interleaved.

                `fillers` is a deque of closures emitting independent PE work
                (e.g. the previous quarter's Wo chains); one is drained per
                inner-loop step to keep the PE busy through the ACT-bound
                softmax stretches."""
                sw = slice(qi * QW, (qi + 1) * QW)
                qT, kT = QK[l]
                if l not in CTX:
                    CTX[l] = [ctxp.tile([128, s], F16, tag="ctxT",
                                        name=f"ctxT{l}_{m}") for m in range(MQ)]
                ctxT = CTX[l]
                for m in range(MQ):          # pair m: heads 2m (off 0), 2m+1 (off 64)
                    heads = (2 * m, 2 * m + 1)
                    pctx = {}
                    probs = {h: [None] * NTP for h in heads}

                    def ctx_mm(h, tp):
                        for half in range(2):
                            t = 2 * tp + half
                            nc.tensor.matmul(
                                pctx[h][:],
                                V[l][t][:, h, :],
                                probs[h][tp][:, half * QW:(half + 1) * QW],
                                start=(t == 0),
                                stop=(t == NT - 1),
                            )

                    for h in heads:
                        pctx[h] = pb.tile([65, QW], F32, tag="pb",
                                          name=f"pctx{l}_{qi}_{h}")
                    for tp in range(NTP):
                        for h in heads:
                            off = 64 * (h % 2)
                            ss = ps2.tile([128, 2 * QW], F32, tag="ss",
                                          name=f"ss{l}_{qi}_{h}_{tp}")
                            for half in range(2):
                                t = 2 * tp + half
                                nc.tensor.matmul(
                                    ss[:, half * QW:(half + 1) * QW],
                                    kT[m][off:off + 64, t * 128:(t + 1) * 128],
                                    qT[m][off:off + 64, sw],
                                    start=True,
                                    stop=True,
                                )
                            pr = prp.tile([128, 2 * QW], F16, tag="probs",
                                          name=f"pr{l}_{qi}_{h}_{tp}")
                            nc.scalar.activation(
                                out=pr[:],
                                in_=ss[:],
                                func=mybir.ActivationFunctionType.Exp,
                                scale=float(1.0 / np.sqrt(HD)),
                            )
                            probs[h][tp] = pr
                        if tp >= LAG:
                            for h in heads:
                                ctx_mm(h, tp - LAG)
                        if fillers:
                            fillers.popleft()()
                    for tp in range(NTP - LAG, NTP):
                        for h in heads:
                            ctx_mm(h, tp)

                    # normalize: ctx^T[d, s'] * (1 / l[s']), l at psum row 64
                    for h in heads:
                        off = 64 * (h % 2)
                        if gp_bcast:
                            r_sb = lrp.tile([1, QW], F32, tag="lnrow",
                                            name=f"r{l}_{qi}_{h}")
                            if fast_recip:
                                nc.vector.reciprocal_approx_fast(
                                    out=r_sb[:], in_=pctx[h][64:65, :]
                                )
                            else:
                                nc.vector.reciprocal(r_sb[:], pctx[h][64:65, :])
                            bcs = rrp.tile([64, QW], F32, tag="bcs",
                                           name=f"bcs{l}_{qi}_{h}")
                            nc.gpsimd.partition_broadcast(
                                bcs[:], r_sb[:], channels=64
                            )
                        else:
                            r_sb = lrp.tile([1, QW], F16, tag="lnrow16",
                                            name=f"r{l}_{qi}_{h}")
                            with nc.allow_low_precision(reason="softmax denom"):
                                nc.vector.reciprocal(r_sb[:], pctx[h][64:65, :])
                            bc = pa.tile([128, max(QW, DQ)], F32, tag="pa")
                            nc.tensor.matmul(
                                bc[0:64, 0:QW], ones16[0:1, 0:64], r_sb[:],
                                start=True, stop=True,
                            )
                            bcs = rrp.tile([64, QW], F16, tag="bcs16",
                                           name=f"bcs{l}_{qi}_{h}")
                            nc.vector.tensor_copy(out=bcs[:], in_=bc[0:64, 0:QW])
                        nc.vector.tensor_mul(
                            out=ctxT[m][off:off + 64, sw],
                            in0=pctx[h][0:64, :],
                            in1=bcs[:],
                        )

            def wo_chain(l, qj, c, arin):
                # One Wo-partial chain: 2 matmuls -> fp16 stage -> DMA to DRAM
                swj = slice(qj * QW, (qj + 1) * QW)
                ctxT = CTX[l]
                wo_sb = W[l]["wo"]
                pd = pa.tile([128, max(QW, DQ)], F32, tag="pa",
                             name=f"pd{l}_{qj}_{c}")
                for m in range(MQ):
                    nc.tensor.matmul(
                        pd[:, 0:QW],
                        wo_sb[:, m, c * 128:(c + 1) * 128],
                        ctxT[m][:, swj],
                        start=(m == 0),
                        stop=(m == MQ - 1),
                    )
                ds = dsp.tile([128, QW], F16, tag="dsend",
                              name=f"ds{l}_{qj}_{c}")
                nc.vector.tensor_copy(out=ds[:], in_=pd[:, 0:QW])
                nc.sync.dma_start(arin[c, :, :], ds[:])

            def ar_trigger(l, qj, arin):
                arout = dramp.tile([HC, 128, QW], F16, tag="arout",
                                   name=f"arout{l}_{qj}")
                nc.gpsimd.collective_compute(
                    "AllReduce",
                    mybir.AluOpType.add,
                    replica_groups=quads,
                    ins=[arin[:].opt()],
                    outs=[arout[:].opt()],
                )
                arouts[(l, qj)] = arout

            def delta_ar_fillers(l, qj):
                """Closures for quarter qj's Wo partials + AllReduce trigger."""
                arin = dramp.tile([HC, 128, QW], F16, tag="arin",
                                  name=f"arin{l}_{qj}")
                fs = [lambda c=c: wo_chain(l, qj, c, arin) for c in range(HC)]
                fs.append(lambda: ar_trigger(l, qj, arin))
                return fs

            def emit_delta_ar(l, qj):
                for f in delta_ar_fillers(l, qj):
                    f()

            def ln_quarter(l, qi):
                """out^T = AR(delta) + bo_eff + x^T ; LN; update x^T."""
                last = l == l_layers - 1
                sw = slice(qi * QW, (qi + 1) * QW)
                arout = arouts[(l, qi)]
                lnw_sb = W[l]["lnw"]
                pst = pb.tile([65, QW], F32, tag="pb", name=f"pst{l}_{qi}")
                ots = []
                for c in range(HC):
                    ot = otp.tile([128, QW], F16, tag="outT",
                                  name=f"ot{l}_{qi}_{c}")
                    nc.sync.dma_start(ot[:], arout[c, :, :])
                    nc.vector.scalar_tensor_tensor(
                        out=ot[:],
                        in0=ot[:],
                        scalar=lnw_sb[:, c, 2:3],
                        in1=xT[c][:, sw],
                        op0=mybir.AluOpType.add,
                        op1=mybir.AluOpType.add,
                    )
                    sqt = sqp.tile([128, QW], F16, tag="sq")
                    nc.vector.tensor_mul(out=sqt[:], in0=ot[:], in1=ot[:])
                    nc.tensor.matmul(
                        pst[0:1, :], ones16[:, 0:1], ot[:],
                        start=(c == 0), stop=(c == HC - 1),
                        skip_group_check=True,
                    )
                    nc.tensor.matmul(
                        pst[32:33, :], ones16[:, 0:1], sqt[:],
                        start=(c == 0), stop=(c == HC - 1),
                        skip_group_check=True,
                    )
                    ots.append(ot)
                sumx = lrp.tile([1, QW], F32, tag="lnrow", name=f"sx{l}_{qi}")
                sumsq = lrp.tile([1, QW], F32, tag="lnrow", name=f"sq{l}_{qi}")
                nc.vector.tensor_copy(out=sumx[:], in_=pst[0:1, :])
                nc.vector.tensor_copy(out=sumsq[:], in_=pst[32:33, :])

                m_sb = lrp.tile([1, QW], F32, tag="lnrow", name=f"m{l}_{qi}")
                nc.vector.tensor_scalar_mul(m_sb[:], sumx[:], 1.0 / H)
                m2 = lrp.tile([1, QW], F32, tag="lnrow", name=f"m2{l}_{qi}")
                nc.vector.tensor_mul(m2[:], m_sb[:], m_sb[:])
                var = lrp.tile([1, QW], F32, tag="lnrow", name=f"va{l}_{qi}")
                nc.vector.scalar_tensor_tensor(
                    out=var[:], in0=sumsq[:], scalar=1.0 / H, in1=m2[:],
                    op0=mybir.AluOpType.mult, op1=mybir.AluOpType.subtract,
                )
                # rstd = exp(-0.5*ln(var)) -- Ln/Exp share one ACT table set,
                # so this avoids the Sqrt-set thrash (var >> eps, eps dropped)
                lnv = lrp.tile([1, QW], F32, tag="lnrow", name=f"lv{l}_{qi}")
                nc.scalar.activation(
                    out=lnv[:], in_=var[:],
                    func=mybir.ActivationFunctionType.Ln,
                )
                rstd = lrp.tile([1, QW], F32, tag="lnrow", name=f"rs{l}_{qi}")
                nc.scalar.activation(
                    out=rstd[:], in_=lnv[:],
                    func=mybir.ActivationFunctionType.Exp,
                    scale=-0.5,
                )

                # broadcast stats across partitions, apply, update x^T
                if gp_bcast:
                    mb = rrp.tile([128, QW], F32, tag="mrb", name=f"mb{l}_{qi}")
                    nc.gpsimd.partition_broadcast(mb[:], m_sb[:], channels=128)
                    rb = rrp.tile([128, QW], F32, tag="mrb", name=f"rb{l}_{qi}")
                    nc.gpsimd.partition_broadcast(rb[:], rstd[:], channels=128)
                else:
                    m16 = lrp.tile([1, QW], F16, tag="lnrow16", name=f"m16{l}_{qi}")
                    nc.vector.tensor_copy(out=m16[:], in_=m_sb[:])
                    r16 = lrp.tile([1, QW], F16, tag="lnrow16", name=f"r16{l}_{qi}")
                    nc.vector.tensor_copy(out=r16[:], in_=rstd[:])
                    mb = pa.tile([128, max(QW, DQ)], F32, tag="pa",
                                 name=f"mb{l}_{qi}")
                    nc.tensor.matmul(
                        mb[:, 0:QW], ones16[0:1, :], m16[:], start=True, stop=True
                    )
                    mb = mb[:, 0:QW]
                    rb = pa.tile([128, max(QW, DQ)], F32, tag="pa",
                                 name=f"rb{l}_{qi}")
                    nc.tensor.matmul(
                        rb[:, 0:QW], ones16[0:1, :], r16[:], start=True, stop=True
                    )
                    rb = rb[:, 0:QW]
                for c in range(HC):
                    tmp = ltp.tile([128, QW], F32, tag="lntmp")
                    nc.vector.tensor_sub(out=tmp[:], in0=ots[c][:], in1=mb[:])
                    nc.vector.scalar_tensor_tensor(
                        out=tmp[:], in0=tmp[:],
                        scalar=lnw_sb[:, c, 0:1], in1=rb[:],
                        op0=mybir.AluOpType.mult, op1=mybir.AluOpType.mult,
                    )
                    if last:
                        fo = fop.tile([128, QW], F32, tag="fout")
                        nc.vector.tensor_scalar_add(
                            fo[:], tmp[:], lnw_sb[:, c, 1:2]
                        )
                        nc.sync.dma_start(outx[c, :, sw], fo[:])
                    else:
                        nc.vector.tensor_scalar_add(
                            xT[c][:, sw], tmp[:], lnw_sb[:, c, 1:2]
                        )

            # ---- schedule ----
            load_weights(0)
            for qi in range(4):
                proj_qk_quarter(0, qi)
            proj_v_chunks(0, range(NT))

            from collections import deque

            for l in range(l_layers):
                last = l == l_layers - 1
                if not last:
                    load_weights(l + 1)
                # Wo chains of quarter qi-1 drain as PE fillers inside
                # attention of quarter qi; LN trails attention by 2 quarters
                # so each AllReduce gets ~1.5 quarters to complete.
                for qi in range(4):
                    fillers = deque(delta_ar_fillers(l, qi - 1)) if qi >= 1 \
                        else None
                    attn_quarter(l, qi, fillers)
                    while fillers:
                        fillers.popleft()()
                    if qi >= 2:
                        ln_quarter(l, qi - 2)
                emit_delta_ar(l, 3)
                # tail: next layer's projections fill the last ARs' latency
                if not last:
                    proj_qk_quarter(l + 1, 0)
                    proj_qk_quarter(l + 1, 1)
                    ln_quarter(l, 2)
                    proj_qk_quarter(l + 1, 2)
                    proj_v_chunks(l + 1, range(12))
                    ln_quarter(l, 3)
                    proj_qk_quarter(l + 1, 3)
                    proj_v_chunks(l + 1, range(12, NT))
                else:
                    ln_quarter(l, 2)
                    ln_quarter(l, 3)
                # drop dead references so pools can recycle
                for d in (QK, V, CTX, W):
                    d.pop(l, None)
    nc.compile()
    return nc


def make_in_maps(inputs, s=S, l_layers=L):
    """Host-side sharding: returns one input dict per core."""
    x = np.asarray(inputs["input_tensor"], dtype=np.float32)      # [s, B, H]
    Wq = np.asarray(inputs["Wq"], dtype=np.float32)[:l_layers]
    Wk = np.asarray(inputs["Wk"], dtype=np.float32)[:l_layers]
    Wv = np.asarray(inputs["Wv"], dtype=np.float32)[:l_layers]
    Wo = np.asarray(inputs["Wo"], dtype=np.float32)[:l_layers]
    bq = np.asarray(inputs["bq"], dtype=np.float32)[:l_layers]
    bk = np.asarray(inputs["bk"], dtype=np.float32)[:l_layers]
    bv = np.asarray(inputs["bv"], dtype=np.float32)[:l_layers]
    bo = np.asarray(inputs["bo"], dtype=np.float32)[:l_layers]
    gamma = np.asarray(inputs["gamma"], dtype=np.float32)[:l_layers]
    beta = np.asarray(inputs["beta"], dtype=np.float32)[:l_layers]
    ll = l_layers

    # bv passes through the softmax-weighted sum exactly: fold bv@Wo into bo.
    bo_eff = bo + np.einsum("lh,lhk->lk", bv, Wo)

    def chunkP(a, n_out):
        # [..., n_out*128, inner] -> [..., 128, n_out, inner] feature-chunked
        sh = a.shape
        a = a.reshape(*sh[:-2], n_out, 128, sh[-1])
        return np.moveaxis(a, -3, -2)  # -> [..., 128, n_out, inner]

    in_maps = []
    for core in range(N_CORES):
        g, j = core // 4, core % 4
        cols = slice(DQ * j, DQ * (j + 1))
        xT = np.ascontiguousarray(x[:, g, :].T).reshape(HC, 128, s)
        wq = np.ascontiguousarray(chunkP(Wq[:, :, cols], HC))      # [L,128,HC,DQ]
        wk = np.ascontiguousarray(chunkP(Wk[:, :, cols], HC))
        wv = np.ascontiguousarray(chunkP(Wv[:, :, cols], HC))
        wo = np.ascontiguousarray(chunkP(Wo[:, cols, :], MQ))      # [L,128,MQ,H]
        bqs = bq[:, cols].reshape(ll, MQ, 128).transpose(0, 2, 1)  # [L,128,MQ]
        bks = bk[:, cols].reshape(ll, MQ, 128).transpose(0, 2, 1)
        bqk = np.ascontiguousarray(np.concatenate([bqs, bks], axis=2))
        lnw = np.stack(
            [
                gamma.reshape(ll, HC, 128).transpose(0, 2, 1),
                beta.reshape(ll, HC, 128).transpose(0, 2, 1),
                bo_eff.reshape(ll, HC, 128).transpose(0, 2, 1),
            ],
            axis=3,
        )                                                          # [L,128,HC,3]
        in_maps.append(
            {
                "xT0": xT.astype(np.float16),
                "wq": wq.astype(np.float16),
                "wk": wk.astype(np.float16),
                "wv": wv.astype(np.float16),
                "wo": wo.astype(np.float16),
                "bqk": bqk.astype(np.float32),
                "lnw": np.ascontiguousarray(lnw).astype(np.float32),
            }
        )
    return in_maps


_NC_CACHE = {}


def kernel(**inputs) -> np.ndarray:
    in_maps = make_in_maps(inputs)
    key = (S, L)
    if key not in _NC_CACHE:
        _NC_CACHE[key] = build_bass()
    nc = _NC_CACHE[key]
    res = run_bass_kernel_spmd(nc, in_maps, core_ids=list(range(N_CORES)))
    out = np.empty((S, B, H), dtype=np.float32)
    for g, core in ((0, 0), (1, 4)):
        xt = res.results[core]["outx"].reshape(H, S)
        out[:, g, :] = xt.T
    return out
